# revision 1
# baseline (speedup 1.0000x reference)
"""Axial attention block (B=8, C=512, H=W=128, 8 heads) on 8 Trainium2 cores.

Sharding: data-parallel over batch — one batch element per NeuronCore. Each
core runs both axial passes on its (C, H, W) slice and produces the full
residual sum xs + oh + ow.

Pass structure (all DMA contiguous-run; no strided DRAM access):
  - Pass 1 (HEIGHT attention, sequences along h, one per w): reads x
    (C,W,H) chunks, computes oh tiles in (c, w-chunk, h) layout and writes
    them to a block-tiled DRAM scratch ohT2[hb][c, w, hi] (h = hb*8 + hi).
  - Pass 2 (WIDTH attention, sequences along w, one per h): h-chunk hb reads
    x + xs(f32) chunks and the matching ohT2[hb] block (contiguous), folds
    oh into the f32 residual once per chunk (GpSimd), then out = ow + resid
    per group (VectorE) and writes natural-layout output.

Projections (Q,K,V,O) run in fp8(e4m3) with DoubleRow perf mode: the K=512
contraction is packed as 2 interleaved 128-row halves per matmul (2 DR
matmuls per 512-contraction instead of 4 bf16 ones). Host pre-scales
x by XS and weights by WS to center fp8 magnitudes; the descales fold into
the PSUM-evacuation copies (which exist anyway).

Per-sequence attention (S=128, dh=64): scoresT = K^T.T @ Q^T per head in
(s_k, s_q) layout, parity-split over two PSUM banks; exp on ScalarE (max
subtraction skipped — scaled scores are bounded ~7); denominators via
ones-matmul on TensorE (replicated rows); reciprocal_approx_fast on VectorE
producing bf16; softmax applied to the exp-weights tile (et * rr at DVE 2x
mode) BEFORE AV so the AV output needs only a single plain evacuation; AV
packs all 8 heads into one PSUM bank in (c, s_q) layout; O-projection
batched over 4 sequences.
"""
import os
import numpy as np
import ml_dtypes

P = 128          # partitions
C = 512          # channels
S = 128          # sequence length (H and W)
NCB = C // P     # channel blocks
NH = 8           # heads
DH = C // NH     # head dim
G = 4            # sequences per projection group
HC1 = 16         # w-chunk, height pass
HC2 = 8          # h-chunk, width pass (= hi block size of ohT2)
HB = S // HC2    # number of h blocks
NCORES = 8

_BF16 = ml_dtypes.bfloat16
_F8 = ml_dtypes.float8_e4m3

# numeric / layout config
FP8 = int(os.environ.get("K_FP8", "1"))      # fp8 DoubleRow QKV projections
FP8_O = int(os.environ.get("K_FP8_O", "0"))  # fp8 DoubleRow O projection
RR_BF16 = int(os.environ.get("K_RR_BF16", "1"))  # bf16 reciprocal output
XS = 16.0        # host pre-scale on x (fp8)
WS = 64.0        # host pre-scale on weights (fp8)
OS = 8.0         # on-device pre-scale on ot (fp8 O-proj input)

# schedule-tuning knobs (env-overridable for experiments)
PROJ_BUFS = int(os.environ.get("K_PROJ_BUFS", "2"))
ATTN_BUFS = int(os.environ.get("K_ATTN_BUFS", "2"))
ET_BUFS = int(os.environ.get("K_ET_BUFS", "2"))
QK_BUFS = int(os.environ.get("K_QK_BUFS", "2"))
VT_BUFS = int(os.environ.get("K_VT_BUFS", "2"))
OT_BUFS = int(os.environ.get("K_OT_BUFS", "2"))
RR_BUFS = int(os.environ.get("K_RR_BUFS", "2"))
PO_BUFS = int(os.environ.get("K_PO_BUFS", "2"))
QK_EVAC = os.environ.get("K_QK_EVAC", "act")
VT_EVAC = os.environ.get("K_VT_EVAC", "act")
PO_EVAC = os.environ.get("K_PO_EVAC", "act")
STG1_ENG = os.environ.get("K_STG1_ENG", "dve")

_PROG = None  # cached compiled Bass program


def _build_program(reps=None):
    """reps=None: normal external-I/O program.  reps=R: timing variant —
    internal DRAM I/O (zero-initialized on device), kernel body repeated R
    times in a hardware For_i loop, tiny tick output."""
    from contextlib import ExitStack
    import concourse.tile as tile
    from concourse import bacc, mybir

    f32 = mybir.dt.float32
    bf = mybir.dt.bfloat16
    f8 = mybir.dt.float8e4
    DRM = mybir.MatmulPerfMode.DoubleRow
    Exp = mybir.ActivationFunctionType.Exp
    MUL = mybir.AluOpType.mult
    ADD = mybir.AluOpType.add

    qk_sc = 1.0 / (XS * WS) if FP8 else 1.0
    vt_sc = 1.0 / (XS * WS) if FP8 else 1.0
    stg_sc = 1.0 / (WS * OS) if FP8_O else 1.0
    ot_dt = f8 if FP8_O else bf

    timed = reps is not None
    ext_in = {} if timed else {"kind": "ExternalInput"}
    ext_out = {} if timed else {"kind": "ExternalOutput"}

    nc = bacc.Bacc("TRN2", target_bir_lowering=False, debug=False)

    xf = nc.dram_tensor("xf", [C, S, S], f32, **ext_in).ap()
    qkv_names = ["wq_w", "wk_w", "wv_w", "wq_h", "wk_h", "wv_h"]
    o_names = ["wo_w", "wo_h"]
    if FP8:
        x8 = nc.dram_tensor("x8", [C, S, S], f8, **ext_in).ap()
        xt8 = nc.dram_tensor("xt8", [C, S, S], f8, **ext_in).ap()
        f8_names = qkv_names + (o_names if FP8_O else [])
        bf_names = [] if FP8_O else o_names
        w8t = {n: nc.dram_tensor(n + "8", [C, C], f8, **ext_in).ap()
               for n in f8_names}
        wt = {n: nc.dram_tensor(n, [C, C], bf, **ext_in).ap() for n in bf_names}
    else:
        xbf = nc.dram_tensor("xbf", [C, S, S], bf, **ext_in).ap()
        xtbf = nc.dram_tensor("xtbf", [C, S, S], bf, **ext_in).ap()
        w8t = {}
        wt = {n: nc.dram_tensor(n, [C, C], bf, **ext_in).ap()
              for n in qkv_names + o_names}
    ohT2 = nc.dram_tensor("ohT2", [HB, C, S, HC2], f32).ap()
    out = nc.dram_tensor("out", [C, S, S], f32, **ext_out).ap()
    tick = nc.dram_tensor("tick", [1, P], bf, kind="ExternalOutput").ap() if timed else None

    with tile.TileContext(nc) as tc, ExitStack() as topctx:
        const = topctx.enter_context(tc.tile_pool(name="const", bufs=1))

        # weights resident in SBUF
        w8_sb = {}   # name -> [2 pair tiles [P, 2, C] fp8]
        wb_sb = {}   # name -> [4 ci tiles [P, C] bf16]
        for n, t in w8t.items():
            tiles = []
            src = t.rearrange("(a j k) n -> a k j n", a=2, j=2)
            for pair in range(2):
                tl = const.tile([P, 2, C], f8, tag=f"w8_{n}_{pair}",
                                name=f"w8_{n}_{pair}")
                nc.sync.dma_start(out=tl, in_=src[pair])
                tiles.append(tl)
            w8_sb[n] = tiles
        for n, t in wt.items():
            tiles = []
            for ci in range(NCB):
                tl = const.tile([P, C], bf, tag=f"w_{n}_{ci}", name=f"w_{n}_{ci}")
                nc.sync.dma_start(out=tl, in_=t[ci * P:(ci + 1) * P, :])
                tiles.append(tl)
            wb_sb[n] = tiles
        ones_sb = const.tile([P, P], bf, tag="ones", name="ones")
        nc.vector.memset(ones_sb, 1.0)

        if timed:
            # zero-init the internal DRAM inputs so the timed loop runs on
            # finite data (uninitialized HBM may hold NaN bit patterns)
            zf = const.tile([P, 1024], f32, tag="zf", name="zf")
            nc.vector.memset(zf, 0.0)
            zb = const.tile([P, 1024], bf, tag="zb", name="zb")
            nc.vector.memset(zb, 0.0)
            z8 = const.tile([P, 1024], f8, tag="z8", name="z8")
            nc.vector.memset(z8, 0.0)
            for cb in range(NCB):
                cs = slice(cb * P, (cb + 1) * P)
                for j in range(16):
                    js = slice(j * 8, (j + 1) * 8)
                    nc.sync.dma_start(
                        out=xf[cs, js, :],
                        in_=zf.rearrange("p (a b) -> p a b", a=8))
                    if FP8:
                        nc.sync.dma_start(
                            out=x8[cs, js, :],
                            in_=z8.rearrange("p (a b) -> p a b", a=8))
                        nc.sync.dma_start(
                            out=xt8[cs, js, :],
                            in_=z8.rearrange("p (a b) -> p a b", a=8))
                    else:
                        nc.sync.dma_start(
                            out=xbf[cs, js, :],
                            in_=zb.rearrange("p (a b) -> p a b", a=8))
                        nc.sync.dma_start(
                            out=xtbf[cs, js, :],
                            in_=zb.rearrange("p (a b) -> p a b", a=8))
                for n in w8t:
                    nc.sync.dma_start(out=w8t[n][cs, :], in_=z8[:, 0:C])
                for n in wt:
                    nc.sync.dma_start(out=wt[n][cs, :], in_=zb[:, 0:C])

        def evac(eng, out_ap, in_ap, sc):
            if eng == "act":
                nc.scalar.mul(out_ap, in_ap, sc) if sc != 1.0 else nc.scalar.copy(out_ap, in_ap)
            else:
                if sc != 1.0:
                    nc.vector.tensor_scalar_mul(out_ap, in_ap, sc)
                else:
                    nc.vector.tensor_copy(out_ap, in_ap)

        def recip(rr, r2):
            if RR_BF16:
                from concourse.dve_ops import (
                    RECIPROCAL_APPROX_FAST, RECIP_APPROX_FAST_CONSTS as RC)
                nc.vector._custom_dve(
                    RECIPROCAL_APPROX_FAST, out=rr, in0=r2,
                    s0=RC["s0"], s1=RC["s1"], imm2=RC["imm2"])
            else:
                nc.vector.reciprocal_approx_fast(out=rr, in_=r2)

        def attn_group(src_t, gsl, s0, wq, wk, wv, wo, pools):
            """One group of G sequences -> psum tiles of out-projection
            results, one (P, G*S) tile per c_out block (scaled by WS*OS in
            fp8-O mode)."""
            (qk_pool, vt_pool, ot_pool, et_pool, rr_pool,
             proj_ps, attn_ps, po_ps) = pools

            qt_sb, kt_sb = [], []
            for wmat, dst_list, nm in ((wq, qt_sb, "qt"), (wk, kt_sb, "kt")):
                for co in range(NCB):
                    pp = proj_ps.tile([P, G * S], f32, tag="proj", name="pp")
                    if FP8:
                        for pair in range(2):
                            nc.tensor.matmul(
                                pp,
                                lhsT=wmat[pair][:, :, co * P:(co + 1) * P],
                                rhs=src_t[pair][:, :, gsl, :],
                                start=(pair == 0), stop=(pair == 1),
                                perf_mode=DRM)
                    else:
                        for ci in range(NCB):
                            nc.tensor.matmul(
                                pp,
                                lhsT=wmat[ci][:, co * P:(co + 1) * P],
                                rhs=src_t[ci][:, gsl, :],
                                start=(ci == 0), stop=(ci == NCB - 1))
                    sb_t = qk_pool.tile([P, G * S], bf, tag=f"{nm}{co}", name=f"{nm}{co}")
                    evac(QK_EVAC, sb_t, pp, qk_sc)
                    dst_list.append(sb_t)

            vt_sb = []
            for sq in range(G):
                pv = proj_ps.tile([P, C], f32, tag="proj", name="pv")
                if FP8:
                    for pair in range(2):
                        nc.tensor.matmul(
                            pv,
                            lhsT=src_t[pair][:, :, s0 + sq, :],
                            rhs=wv[pair],
                            start=(pair == 0), stop=(pair == 1),
                            perf_mode=DRM)
                else:
                    for ci in range(NCB):
                        nc.tensor.matmul(
                            pv, lhsT=src_t[ci][:, s0 + sq, :], rhs=wv[ci],
                            start=(ci == 0), stop=(ci == NCB - 1))
                vt = vt_pool.tile([P, C], bf, tag=f"vt{sq}", name=f"vt{sq}")
                evac(VT_EVAC, vt, pv, vt_sc)
                vt_sb.append(vt)

            ot_full = ot_pool.tile([P, NCB, G * S], ot_dt, tag="ot", name="ot")
            for sq in range(G):
                ssl = slice(sq * S, (sq + 1) * S)
                # scoresT: head h -> col h//2*128 of half (h%2); the two
                # 512-col halves are separate PSUM banks, so even (row-group
                # 0-63) and odd (64-127) head matmuls never share a bank
                st2 = attn_ps.tile([P, 1024], f32, tag="attn", name="st2")
                for h in range(NH):
                    par, cb = h % 2, h // 2
                    rows = slice(par * DH, (par + 1) * DH)
                    nc.tensor.matmul(
                        st2[:, par * 512 + cb * S:par * 512 + (cb + 1) * S],
                        lhsT=kt_sb[h // 2][rows, ssl],
                        rhs=qt_sb[h // 2][rows, ssl],
                        start=True, stop=True)
                et = et_pool.tile([P, 1024], bf, tag="et", name="et")
                nc.scalar.activation(out=et, in_=st2, func=Exp, scale=DH ** -0.5)
                r2 = attn_ps.tile([P, 1024], f32, tag="attn", name="r2")
                nc.tensor.matmul(r2[:, 0:512], lhsT=ones_sb, rhs=et[:, 0:512],
                                 start=True, stop=True)
                nc.tensor.matmul(r2[:, 512:1024], lhsT=ones_sb, rhs=et[:, 512:1024],
                                 start=True, stop=True)
                rr = rr_pool.tile([P, 1024], bf if RR_BF16 else f32,
                                  tag="rr", name="rr")
                recip(rr, r2)
                # AV on unnormalized exp; softmax denominators commute past
                # the matmul (pure column scaling), so recip runs on VectorE
                # in parallel with AV on TensorE and the normalize fuses into
                # the psum->sbuf evacuation below.
                po = po_ps.tile([P, 512], f32, tag="po", name="po")
                for h in range(NH):
                    par, cb = h % 2, h // 2
                    nc.tensor.matmul(
                        po[par * DH:(par + 1) * DH, cb * S:(cb + 1) * S],
                        lhsT=vt_sb[sq][:, h * DH:(h + 1) * DH],
                        rhs=et[:, par * 512 + cb * S:par * 512 + (cb + 1) * S],
                        start=True, stop=True)
                # row-half r of po holds heads with parity r; its per-element
                # normalizer is exactly rr[:, r*512:] (rows replicated).
                # po rows par*64+d, col block cb hold head h=2cb+par →
                # channel c = cb*128 + par*64 + d: po maps to ot channel-major
                if FP8_O:
                    nc.vector.scalar_tensor_tensor(
                        out=ot_full[0:DH, :, ssl],
                        in0=po[0:DH, :].rearrange("p (c s) -> p c s", c=NCB),
                        scalar=OS,
                        in1=rr[0:DH, 0:512].rearrange("p (c s) -> p c s", c=NCB),
                        op0=MUL, op1=MUL)
                    nc.vector.scalar_tensor_tensor(
                        out=ot_full[DH:P, :, ssl],
                        in0=po[DH:P, :].rearrange("p (c s) -> p c s", c=NCB),
                        scalar=OS,
                        in1=rr[DH:P, 512:1024].rearrange("p (c s) -> p c s", c=NCB),
                        op0=MUL, op1=MUL)
                else:
                    nc.vector.tensor_mul(
                        ot_full[0:DH, :, ssl],
                        po[0:DH, :].rearrange("p (c s) -> p c s", c=NCB),
                        rr[0:DH, 0:512].rearrange("p (c s) -> p c s", c=NCB))
                    nc.vector.tensor_mul(
                        ot_full[DH:P, :, ssl],
                        po[DH:P, :].rearrange("p (c s) -> p c s", c=NCB),
                        rr[DH:P, 512:1024].rearrange("p (c s) -> p c s", c=NCB))
            # O-projection outputs go through the attn pool's 2-bank tiles
            # (pairs of c_out blocks in the two bank halves) so the proj pool
            # frees up for the next group's Q/K/V immediately
            pods = []
            if FP8_O:
                otv = ot_full.rearrange("p (a j) s -> p a j s", a=2)
            for cop in range(NCB // 2):
                pp2 = attn_ps.tile([P, 1024], f32, tag="attn", name="pp2")
                for half in range(2):
                    co = cop * 2 + half
                    dst = pp2[:, half * 512:(half + 1) * 512]
                    if FP8_O:
                        for pair in range(2):
                            nc.tensor.matmul(
                                dst,
                                lhsT=wo[pair][:, :, co * P:(co + 1) * P],
                                rhs=otv[:, pair],
                                start=(pair == 0), stop=(pair == 1),
                                perf_mode=DRM)
                    else:
                        for ci in range(NCB):
                            nc.tensor.matmul(
                                dst,
                                lhsT=wo[ci][:, co * P:(co + 1) * P],
                                rhs=ot_full[:, ci, :],
                                start=(ci == 0), stop=(ci == NCB - 1))
                    pods.append(dst)
            return pods

        def get_w(kind, suffix):
            n = f"w{kind}_{suffix}"
            if kind in ("q", "k", "v"):
                return w8_sb[n] if FP8 else wb_sb[n]
            return w8_sb[n] if (FP8 and FP8_O) else wb_sb[n]

        def mk_pools(ctx, sfx):
            qk_pool = ctx.enter_context(tc.tile_pool(name=f"qk{sfx}", bufs=QK_BUFS))
            vt_pool = ctx.enter_context(tc.tile_pool(name=f"vt{sfx}", bufs=VT_BUFS))
            ot_pool = ctx.enter_context(tc.tile_pool(name=f"ot{sfx}", bufs=OT_BUFS))
            et_pool = ctx.enter_context(tc.tile_pool(name=f"et{sfx}", bufs=ET_BUFS))
            rr_pool = ctx.enter_context(tc.tile_pool(name=f"rr{sfx}", bufs=RR_BUFS))
            proj_ps = ctx.enter_context(
                tc.tile_pool(name=f"pps{sfx}", bufs=PROJ_BUFS, space="PSUM"))
            attn_ps = ctx.enter_context(
                tc.tile_pool(name=f"aps{sfx}", bufs=ATTN_BUFS, space="PSUM"))
            po_ps = ctx.enter_context(
                tc.tile_pool(name=f"pops{sfx}", bufs=PO_BUFS, space="PSUM"))
            return (qk_pool, vt_pool, ot_pool, et_pool, rr_pool,
                    proj_ps, attn_ps, po_ps)

        def load_src(src_pool, xsrc, q0, hc):
            """DMA the x chunk for [q0, q0+hc) sequences into SBUF tiles."""
            src_t = []
            if FP8:
                xv = xsrc.rearrange("(a j k) w h -> a k j w h", a=2, j=2)
                for pair in range(2):
                    t = src_pool.tile([P, 2, hc, S], f8, tag=f"src{pair}",
                                      name=f"src{pair}")
                    nc.sync.dma_start(out=t, in_=xv[pair, :, :, q0:q0 + hc, :])
                    src_t.append(t)
            else:
                for cb in range(NCB):
                    cs = slice(cb * P, (cb + 1) * P)
                    t = src_pool.tile([P, hc, S], bf, tag=f"src{cb}",
                                      name=f"src{cb}")
                    nc.sync.dma_start(out=t, in_=xsrc[cs, q0:q0 + hc, :])
                    src_t.append(t)
            return src_t

        def height_pass():
            """Pass 1: height attention (seq along h, one per w).  Writes oh
            to the blocked scratch ohT2[hb][c, w, hi]."""
            wq, wk, wv, wo = (get_w("q", "h"), get_w("k", "h"),
                              get_w("v", "h"), get_w("o", "h"))
            with ExitStack() as ctx:
                src_pool = ctx.enter_context(tc.tile_pool(name="src1", bufs=2))
                stage_pool = ctx.enter_context(tc.tile_pool(name="stg1", bufs=2))
                pools = mk_pools(ctx, "1")

                for chunk in range(S // HC1):
                    q0 = chunk * HC1
                    if not FP8:
                        src_t = load_src(src_pool, xtbf, q0, HC1)
                    stage_t = []
                    for cb in range(NCB):
                        # stage layout (hb, w, hi): contiguous runs on both
                        # DMA sides of the blocked write
                        st = stage_pool.tile([P, HB, HC1, HC2], f32,
                                             tag=f"stg{cb}", name=f"stg{cb}")
                        stage_t.append(st)
                    for g in range(HC1 // G):
                        s0 = g * G
                        gsl = slice(s0, s0 + G)
                        if FP8:
                            # per-group contiguous tile: attn_group sees
                            # tile-relative indices; stage keeps gsl
                            src_t = load_src(src_pool, xt8, q0 + s0, G)
                            agsl, as0 = slice(0, G), 0
                        else:
                            agsl, as0 = gsl, s0
                        pods = attn_group(src_t, agsl, as0, wq, wk, wv, wo, pools)
                        for co in range(NCB):
                            # pods: (p, 4 w-seq, 128 h) -> stage (hb, w, hi)
                            evac(STG1_ENG,
                                 stage_t[co][:, :, gsl, :].rearrange("p b q i -> p q b i"),
                                 pods[co].rearrange("p (q b i) -> p q b i", q=G, b=HB),
                                 stg_sc)
                    for cb in range(NCB):
                        cs = slice(cb * P, (cb + 1) * P)
                        nc.sync.dma_start(
                            out=ohT2[:, cs, q0:q0 + HC1, :].rearrange("b c w i -> c b w i"),
                            in_=stage_t[cb])

        def width_pass():
            """Pass 2: width attention (seq along w, one per h).  h-chunk =
            hb block; out = xs + oh + ow in natural layout."""
            wq, wk, wv, wo = (get_w("q", "w"), get_w("k", "w"),
                              get_w("v", "w"), get_w("o", "w"))
            with ExitStack() as ctx:
                src_pool = ctx.enter_context(tc.tile_pool(name="src2", bufs=2))
                resid_pool = ctx.enter_context(tc.tile_pool(name="res2", bufs=2))
                oh_pool = ctx.enter_context(tc.tile_pool(name="oh2", bufs=2))
                stage_pool = ctx.enter_context(tc.tile_pool(name="stg2", bufs=2))
                pools = mk_pools(ctx, "2")

                for hb in range(HB):
                    q0 = hb * HC2
                    if not FP8:
                        src_t = load_src(src_pool, xbf, q0, HC2)
                    resid_t, stage_t = [], []
                    for cb in range(NCB):
                        cs = slice(cb * P, (cb + 1) * P)
                        rt = resid_pool.tile([P, HC2, S], f32, tag=f"res{cb}", name=f"res{cb}")
                        nc.sync.dma_start(out=rt, in_=xf[cs, q0:q0 + HC2, :])
                        resid_t.append(rt)
                        oht = oh_pool.tile([P, S, HC2], f32, tag=f"oh{cb}", name=f"oh{cb}")
                        nc.sync.dma_start(out=oht, in_=ohT2[hb, cs, :, :])
                        # fold oh into the residual once per chunk
                        nc.gpsimd.tensor_tensor(
                            out=rt, in0=rt,
                            in1=oht.rearrange("p w i -> p i w"),
                            op=mybir.AluOpType.add)
                        st = stage_pool.tile([P, HC2, S], f32, tag=f"stg{cb}", name=f"stg{cb}")
                        stage_t.append(st)
                    for g in range(HC2 // G):
                        s0 = g * G
                        gsl = slice(s0, s0 + G)
                        if FP8:
                            src_t = load_src(src_pool, x8, q0 + s0, G)
                            agsl, as0 = slice(0, G), 0
                        else:
                            agsl, as0 = gsl, s0
                        pods = attn_group(src_t, agsl, as0, wq, wk, wv, wo, pools)
                        for co in range(NCB):
                            src_ap = pods[co].rearrange("p (q s) -> p q s", q=G)
                            if stg_sc != 1.0:
                                nc.vector.scalar_tensor_tensor(
                                    out=stage_t[co][:, gsl, :],
                                    in0=src_ap, scalar=stg_sc,
                                    in1=resid_t[co][:, gsl, :],
                                    op0=MUL, op1=ADD)
                            else:
                                nc.vector.tensor_add(
                                    stage_t[co][:, gsl, :],
                                    src_ap, resid_t[co][:, gsl, :])
                    for cb in range(NCB):
                        cs = slice(cb * P, (cb + 1) * P)
                        nc.sync.dma_start(out=out[cs, q0:q0 + HC2, :], in_=stage_t[cb])

        if timed:
            with tc.For_i(0, reps, 1):
                height_pass()
                width_pass()
            nc.sync.dma_start(out=tick, in_=ones_sb[0:1, :])
        else:
            height_pass()
            width_pass()

    nc.compile()
    return nc


def _get_program():
    global _PROG
    if _PROG is None:
        _PROG = _build_program()
    return _PROG


def _host_prep(xs, wmap):
    """Per-batch input maps for the SPMD run."""
    xs = np.asarray(xs, dtype=np.float32)
    base = {}
    if FP8:
        for n, w in wmap.items():
            wt8 = np.ascontiguousarray(np.asarray(w, dtype=np.float32).T) * WS
            if n.startswith("wo") and not FP8_O:
                base[n] = (wt8 / WS).astype(_BF16)
            else:
                base[n + "8"] = wt8.astype(_F8)
    else:
        for n, w in wmap.items():
            base[n] = np.ascontiguousarray(
                np.asarray(w, dtype=np.float32).T).astype(_BF16)

    in_maps = []
    for b in range(NCORES):
        xb = np.ascontiguousarray(xs[b])                        # (C, H, W) f32
        m = {"xf": xb, **base}
        if FP8:
            xss = xb * XS
            m["x8"] = xss.astype(_F8)
            m["xt8"] = np.ascontiguousarray(np.swapaxes(xss, 1, 2)).astype(_F8)
        else:
            m["xbf"] = xb.astype(_BF16)
            m["xtbf"] = np.ascontiguousarray(np.swapaxes(xb, 1, 2)).astype(_BF16)
        in_maps.append(m)
    return in_maps


def kernel(xs, Wq_h, Wk_h, Wv_h, Wo_h, Wq_w, Wk_w, Wv_w, Wo_w):
    from concourse.bass_utils import run_bass_kernel_spmd

    nc = _get_program()

    wmap = {
        "wq_w": Wq_w, "wk_w": Wk_w, "wv_w": Wv_w, "wo_w": Wo_w,
        "wq_h": Wq_h, "wk_h": Wk_h, "wv_h": Wv_h, "wo_h": Wo_h,
    }
    in_maps = _host_prep(xs, wmap)
    res = run_bass_kernel_spmd(nc, in_maps, core_ids=list(range(NCORES)))
    return np.stack([res.results[b]["out"] for b in range(NCORES)], axis=0)



# revision 4
# speedup vs baseline: 1.3655x; 1.3655x over previous
"""Axial attention block (B=8, C=512, H=W=128, 8 heads) on 8 Trainium2 cores.

Sharding: data-parallel over batch — one batch element per NeuronCore. Each
core runs both axial passes on its (C, H, W) slice and produces the full
residual sum xs + oh + ow.

v3 design (HBM-traffic-minimized, engine-balanced, PE-pipelined):
  - oh never round-trips to DRAM: pass 1 writes (oh + xs) into a
    SBUF-resident f16 buffer ohsb[co] [P, w, h]; pass 2 reads it back
    transposed as the residual for the final sum.  DRAM traffic per core:
    xt8 + x8 (fp8, 16.8 MB), xt16 (f16 transposed xs, 16.8 MB), out16
    (f16 output, 16.8 MB) ~50 MB vs 150 MB in v1 — the 8 cores contend
    for shared HBM, so traffic dominates at 8-core scale.
  - Softmax denominators in a parity-split PSUM layout: two ones-matmuls
    write denom rows into partitions 0:64 (even heads) / 64:128 (odd
    heads) of one [P,512] tile, so reciprocal and normalize run once at
    full width.
  - PSUM rings sized to exactly 8 banks: proj [P,512]x3, scores/denoms
    shared ring [P,512]x3 (st2a, st2b, r2 per seq), AV/O-proj shared ring
    [P,512]x2 (po per seq, pod per c_out).
  - exp split into parity halves so AV of even heads starts ~600 ns
    earlier; evacuations spread across ScalarE/VectorE/GpSimd so no
    elementwise engine exceeds TensorE busy time.
  - Software pipelining: group n+1's Q/K/V projections are emitted
    between group n's attention tail and its O-projection, hiding the
    exp/AV/normalize latency of the last sequences.
"""
import os
import numpy as np
import ml_dtypes

P = 128          # partitions
C = 512          # channels
S = 128          # sequence length (H and W)
NCB = C // P     # channel blocks
NH = 8           # heads
DH = C // NH     # head dim
G = 4            # sequences per group
NG = S // G      # groups per pass
NCORES = 8

_BF16 = ml_dtypes.bfloat16
_F8 = ml_dtypes.float8_e4m3

FP8_O = int(os.environ.get("K_FP8_O", "0"))  # fp8 DoubleRow O projection
XS = 16.0        # host pre-scale on x (fp8)
WS = 64.0        # host pre-scale on weights (fp8)
OS = 8.0         # on-device pre-scale on ot (fp8 O-proj input)

_PROG = None  # cached compiled Bass program


def _build_program(reps=None):
    """reps=None: normal external-I/O program.  reps=R: timing variant —
    internal DRAM I/O (zero-initialized on device), kernel body repeated R
    times in a hardware For_i loop, tiny tick output."""
    from contextlib import ExitStack
    import concourse.tile as tile
    from concourse import bacc, mybir

    f32 = mybir.dt.float32
    bf = mybir.dt.bfloat16
    f16 = mybir.dt.float16
    f8 = mybir.dt.float8e4
    DRM = mybir.MatmulPerfMode.DoubleRow
    Exp = mybir.ActivationFunctionType.Exp
    MUL = mybir.AluOpType.mult
    ADD = mybir.AluOpType.add

    qk_sc = 1.0 / (XS * WS)
    vt_sc = 1.0 / (XS * WS)
    stg_sc = 1.0 / (WS * OS) if FP8_O else 1.0
    ot_dt = f8 if FP8_O else bf

    timed = reps is not None
    ext_in = {} if timed else {"kind": "ExternalInput"}

    nc = bacc.Bacc("TRN2", target_bir_lowering=False, debug=False)

    x8 = nc.dram_tensor("x8", [C, S, S], f8, **ext_in).ap()      # (C,H,W)
    xt8 = nc.dram_tensor("xt8", [C, S, S], f8, **ext_in).ap()    # (C,W,H)
    xt16 = nc.dram_tensor("xt16", [C, S, S], f16, **ext_in).ap() # (C,W,H)
    qkv_names = ["wq_w", "wk_w", "wv_w", "wq_h", "wk_h", "wv_h"]
    o_names = ["wo_w", "wo_h"]
    f8_names = qkv_names + (o_names if FP8_O else [])
    bf_names = [] if FP8_O else o_names
    w8t = {n: nc.dram_tensor(n + "8", [C, C], f8, **ext_in).ap()
           for n in f8_names}
    wt = {n: nc.dram_tensor(n, [C, C], bf, **ext_in).ap() for n in bf_names}
    if timed:
        out16 = nc.dram_tensor("out16", [C, S, S], f16).ap()
    else:
        out16 = nc.dram_tensor("out16", [C, S, S], f16,
                               kind="ExternalOutput").ap()
    tick = nc.dram_tensor("tick", [1, P], bf, kind="ExternalOutput").ap() if timed else None

    # transposed DRAM views: partition = channel-within-block
    xt8v = xt8.rearrange("(a j k) w h -> a k j w h", a=2, j=2)
    x8v = x8.rearrange("(a j k) h w -> a k j h w", a=2, j=2)
    xt16v = xt16.rearrange("(c k) w h -> k c w h", c=NCB)
    out16v = out16.rearrange("(c k) h w -> k c h w", c=NCB)

    with tile.TileContext(nc) as tc, ExitStack() as topctx:
        const = topctx.enter_context(tc.tile_pool(name="const", bufs=1))

        # weights resident in SBUF
        w8_sb = {}   # name -> [2 pair tiles [P, 2, C] fp8]
        wb_sb = {}   # name -> [4 ci tiles [P, C] bf16]
        for n, t in w8t.items():
            tiles = []
            src = t.rearrange("(a j k) n -> a k j n", a=2, j=2)
            for pair in range(2):
                tl = const.tile([P, 2, C], f8, tag=f"w8_{n}_{pair}",
                                name=f"w8_{n}_{pair}")
                nc.sync.dma_start(out=tl, in_=src[pair])
                tiles.append(tl)
            w8_sb[n] = tiles
        for n, t in wt.items():
            tiles = []
            for ci in range(NCB):
                tl = const.tile([P, C], bf, tag=f"w_{n}_{ci}", name=f"w_{n}_{ci}")
                nc.sync.dma_start(out=tl, in_=t[ci * P:(ci + 1) * P, :])
                tiles.append(tl)
            wb_sb[n] = tiles
        ones_sb = const.tile([P, P], bf, tag="ones", name="ones")
        nc.vector.memset(ones_sb, 1.0)

        # SBUF-resident oh+xs accumulator, f16, one tile per c_out block
        ohsb = []
        for co in range(NCB):
            t = const.tile([P, S, S], f16, tag=f"ohsb{co}", name=f"ohsb{co}")
            ohsb.append(t)

        if timed:
            zb = const.tile([P, 1024], bf, tag="zb", name="zb")
            nc.vector.memset(zb, 0.0)
            z16 = const.tile([P, 1024], f16, tag="z16", name="z16")
            nc.vector.memset(z16, 0.0)
            z8 = const.tile([P, 1024], f8, tag="z8", name="z8")
            nc.vector.memset(z8, 0.0)
            for cb in range(NCB):
                cs = slice(cb * P, (cb + 1) * P)
                for j in range(16):
                    js = slice(j * 8, (j + 1) * 8)
                    nc.sync.dma_start(
                        out=x8[cs, js, :],
                        in_=z8.rearrange("p (a b) -> p a b", a=8))
                    nc.sync.dma_start(
                        out=xt8[cs, js, :],
                        in_=z8.rearrange("p (a b) -> p a b", a=8))
                    nc.sync.dma_start(
                        out=xt16[cs, js, :],
                        in_=z16.rearrange("p (a b) -> p a b", a=8))
                for n in w8t:
                    nc.sync.dma_start(out=w8t[n][cs, :], in_=z8[:, 0:C])
                for n in wt:
                    nc.sync.dma_start(out=wt[n][cs, :], in_=zb[:, 0:C])

        # elementwise-engine dispatch helpers -----------------------------
        def ew_scale(eng, out_ap, in_ap, sc):
            if eng == "act":
                nc.scalar.mul(out_ap, in_ap, sc) if sc != 1.0 else nc.scalar.copy(out_ap, in_ap)
            elif eng == "dve":
                if sc != 1.0:
                    nc.vector.tensor_scalar_mul(out_ap, in_ap, sc)
                else:
                    nc.vector.tensor_copy(out_ap, in_ap)
            else:
                if sc != 1.0:
                    nc.gpsimd.tensor_scalar_mul(out_ap, in_ap, sc)
                else:
                    nc.gpsimd.tensor_copy(out_ap, in_ap)

        def ew_stage(eng, out_ap, in0_ap, in1_ap):
            """out = in0 * stg_sc + in1 on a chosen engine."""
            mod = {"dve": nc.vector, "pool": nc.gpsimd}[eng]
            if stg_sc != 1.0:
                mod.scalar_tensor_tensor(out=out_ap, in0=in0_ap, scalar=stg_sc,
                                         in1=in1_ap, op0=MUL, op1=ADD)
            else:
                mod.tensor_add(out_ap, in0_ap, in1_ap)

        # GpSimd/Pool has no PSUM port, so every PSUM-reading op must run on
        # ScalarE (act) or VectorE (dve).  tensor_tensor ops (stage adds,
        # normalize) are DVE-only; exp is Act-only; the plain evacuations
        # are split to balance the two queues.
        QK_ENG = ["act"] * 8
        VT_ENG = ["act", "dve", "dve", "dve"]
        STG_ENG = ["dve", "dve", "dve", "dve"]

        def recip(rr, r2):
            from concourse.dve_ops import (
                RECIPROCAL_APPROX_FAST, RECIP_APPROX_FAST_CONSTS as RC)
            nc.vector._custom_dve(
                RECIPROCAL_APPROX_FAST, out=rr, in0=r2,
                s0=RC["s0"], s1=RC["s1"], imm2=RC["imm2"])

        def proj_phase(src_t, wq, wk, wv, pools):
            """Q, K, V projections (fp8 DoubleRow) for one group."""
            (qk_pool, vt_pool, ot_pool, et_pool, rr_pool,
             proj_ps, sr_ps, po_ps) = pools
            qt_sb, kt_sb = [], []
            for wmat, dst_list, nm in ((wq, qt_sb, "qt"), (wk, kt_sb, "kt")):
                for co in range(NCB):
                    pp = proj_ps.tile([P, G * S], f32, tag="proj", name="pp")
                    for pair in range(2):
                        nc.tensor.matmul(
                            pp,
                            lhsT=wmat[pair][:, :, co * P:(co + 1) * P],
                            rhs=src_t[pair],
                            start=(pair == 0), stop=(pair == 1),
                            perf_mode=DRM)
                    sb_t = qk_pool.tile([P, G * S], bf, tag=f"{nm}{co}",
                                        name=f"{nm}{co}")
                    ew_scale(QK_ENG[(0 if nm == "qt" else NCB) + co],
                             sb_t, pp, qk_sc)
                    dst_list.append(sb_t)
            vt_sb = []
            for sq in range(G):
                pv = proj_ps.tile([P, C], f32, tag="proj", name="pv")
                for pair in range(2):
                    nc.tensor.matmul(
                        pv,
                        lhsT=src_t[pair][:, :, sq, :],
                        rhs=wv[pair],
                        start=(pair == 0), stop=(pair == 1),
                        perf_mode=DRM)
                vt = vt_pool.tile([P, C], bf, tag=f"vt{sq}", name=f"vt{sq}")
                ew_scale(VT_ENG[sq], vt, pv, vt_sc)
                vt_sb.append(vt)
            return qt_sb, kt_sb, vt_sb

        def attn_phase(qkv, pools):
            """Scores, softmax, AV for all G sequences -> normalized ot tile."""
            qt_sb, kt_sb, vt_sb = qkv
            (qk_pool, vt_pool, ot_pool, et_pool, rr_pool,
             proj_ps, sr_ps, po_ps) = pools
            ot_full = ot_pool.tile([P, NCB, G * S], ot_dt, tag="ot", name="ot")
            for sq in range(G):
                ssl = slice(sq * S, (sq + 1) * S)
                et = et_pool.tile([P, 1024], bf, tag="et", name="et")
                # even heads h=2cb: rows 0:64 of block cb; odd: rows 64:128
                st2a = sr_ps.tile([P, 512], f32, tag="sr", name="st2a")
                for cb in range(NCB):
                    nc.tensor.matmul(
                        st2a[:, cb * S:(cb + 1) * S],
                        lhsT=kt_sb[cb][0:DH, ssl],
                        rhs=qt_sb[cb][0:DH, ssl],
                        start=True, stop=True)
                nc.scalar.activation(out=et[:, 0:512], in_=st2a, func=Exp,
                                     scale=DH ** -0.5)
                st2b = sr_ps.tile([P, 512], f32, tag="sr", name="st2b")
                for cb in range(NCB):
                    nc.tensor.matmul(
                        st2b[:, cb * S:(cb + 1) * S],
                        lhsT=kt_sb[cb][DH:P, ssl],
                        rhs=qt_sb[cb][DH:P, ssl],
                        start=True, stop=True)
                nc.scalar.activation(out=et[:, 512:1024], in_=st2b, func=Exp,
                                     scale=DH ** -0.5)
                # denominators, parity-split over partition halves
                r2 = sr_ps.tile([P, 512], f32, tag="sr", name="r2")
                nc.tensor.matmul(r2[0:DH, :], lhsT=ones_sb[:, 0:DH],
                                 rhs=et[:, 0:512], start=True, stop=True)
                nc.tensor.matmul(r2[DH:P, :], lhsT=ones_sb[:, 0:DH],
                                 rhs=et[:, 512:1024], start=True, stop=True)
                rr = rr_pool.tile([P, 512], bf, tag="rr", name="rr")
                recip(rr, r2)
                # AV on unnormalized exp; normalize fuses into the
                # psum->sbuf evacuation (recip overlaps AV on TensorE)
                po = po_ps.tile([P, 512], f32, tag="po", name="po")
                for h in range(NH):
                    par, cb = h % 2, h // 2
                    nc.tensor.matmul(
                        po[par * DH:(par + 1) * DH, cb * S:(cb + 1) * S],
                        lhsT=vt_sb[sq][:, h * DH:(h + 1) * DH],
                        rhs=et[:, par * 512 + cb * S:par * 512 + (cb + 1) * S],
                        start=True, stop=True)
                # po row par*64+d, col block cb holds head h=2cb+par ->
                # normalizer rr[p, cb*128+s] has matching parity by p-half
                if FP8_O:
                    nc.vector.scalar_tensor_tensor(
                        out=ot_full[:, :, ssl],
                        in0=po.rearrange("p (c s) -> p c s", c=NCB),
                        scalar=OS,
                        in1=rr.rearrange("p (c s) -> p c s", c=NCB),
                        op0=MUL, op1=MUL)
                else:
                    nc.vector.tensor_mul(
                        ot_full[:, :, ssl],
                        po.rearrange("p (c s) -> p c s", c=NCB),
                        rr.rearrange("p (c s) -> p c s", c=NCB))
            return ot_full

        def oproj_phase(ot_full, wo, pools):
            """O-projection -> 4 psum tiles [P, G*S], one per c_out block."""
            (qk_pool, vt_pool, ot_pool, et_pool, rr_pool,
             proj_ps, sr_ps, po_ps) = pools
            pods = []
            if FP8_O:
                otv = ot_full.rearrange("p (a j) s -> p a j s", a=2)
            for co in range(NCB):
                pod = po_ps.tile([P, 512], f32, tag="po", name="pod")
                if FP8_O:
                    for pair in range(2):
                        nc.tensor.matmul(
                            pod,
                            lhsT=wo[pair][:, :, co * P:(co + 1) * P],
                            rhs=otv[:, pair],
                            start=(pair == 0), stop=(pair == 1),
                            perf_mode=DRM)
                else:
                    for ci in range(NCB):
                        nc.tensor.matmul(
                            pod,
                            lhsT=wo[ci][:, co * P:(co + 1) * P],
                            rhs=ot_full[:, ci, :],
                            start=(ci == 0), stop=(ci == NCB - 1))
                pods.append(pod)
            return pods

        def get_w(kind, suffix):
            n = f"w{kind}_{suffix}"
            if kind in ("q", "k", "v"):
                return w8_sb[n]
            return w8_sb[n] if FP8_O else wb_sb[n]

        def mk_pools(ctx, sfx):
            qk_pool = ctx.enter_context(tc.tile_pool(name=f"qk{sfx}", bufs=2))
            vt_pool = ctx.enter_context(tc.tile_pool(name=f"vt{sfx}", bufs=2))
            ot_pool = ctx.enter_context(tc.tile_pool(name=f"ot{sfx}", bufs=2))
            et_pool = ctx.enter_context(tc.tile_pool(name=f"et{sfx}", bufs=2))
            rr_pool = ctx.enter_context(tc.tile_pool(name=f"rr{sfx}", bufs=2))
            proj_ps = ctx.enter_context(
                tc.tile_pool(name=f"pps{sfx}", bufs=3, space="PSUM"))
            sr_ps = ctx.enter_context(
                tc.tile_pool(name=f"srs{sfx}", bufs=3, space="PSUM"))
            po_ps = ctx.enter_context(
                tc.tile_pool(name=f"pos{sfx}", bufs=2, space="PSUM"))
            return (qk_pool, vt_pool, ot_pool, et_pool, rr_pool,
                    proj_ps, sr_ps, po_ps)

        def load_src(src_pool, xsrcv, q0):
            src_t = []
            for pair in range(2):
                t = src_pool.tile([P, 2, G, S], f8, tag=f"src{pair}",
                                  name=f"src{pair}")
                nc.sync.dma_start(out=t, in_=xsrcv[pair, :, :, q0:q0 + G, :])
                src_t.append(t)
            return src_t

        def run_pass(is_height):
            """Software-pipelined group loop for one axial pass."""
            sfx = "1" if is_height else "2"
            suffix = "h" if is_height else "w"
            wq, wk, wv, wo = (get_w("q", suffix), get_w("k", suffix),
                              get_w("v", suffix), get_w("o", suffix))
            with ExitStack() as ctx:
                src_pool = ctx.enter_context(
                    tc.tile_pool(name=f"src{sfx}", bufs=2))
                if is_height:
                    xr_pool = ctx.enter_context(
                        tc.tile_pool(name="xr1", bufs=2))
                else:
                    stage_pool = ctx.enter_context(
                        tc.tile_pool(name="stg2", bufs=2))
                pools = mk_pools(ctx, sfx)

                def start_group(g):
                    q0 = g * G
                    src_t = load_src(src_pool, xt8v if is_height else x8v, q0)
                    xr = None
                    if is_height:
                        xr = xr_pool.tile([P, NCB, G, S], f16, tag="xr",
                                          name="xr")
                        nc.sync.dma_start(out=xr,
                                          in_=xt16v[:, :, q0:q0 + G, :])
                    qkv = proj_phase(src_t, wq, wk, wv, pools)
                    return qkv, xr

                def finish_group(g, ot_full, xr):
                    q0 = g * G
                    pods = oproj_phase(ot_full, wo, pools)
                    if is_height:
                        for co in range(NCB):
                            ew_stage(
                                STG_ENG[co],
                                ohsb[co][:, q0:q0 + G, :],
                                pods[co].rearrange("p (q s) -> p q s", q=G),
                                xr[:, co])
                    else:
                        st = stage_pool.tile([P, NCB, G, S], f16, tag="st",
                                             name="st")
                        for co in range(NCB):
                            ew_stage(
                                STG_ENG[co],
                                st[:, co],
                                pods[co].rearrange("p (q s) -> p q s", q=G),
                                ohsb[co][:, :, q0:q0 + G].rearrange(
                                    "p w i -> p i w"))
                        nc.sync.dma_start(out=out16v[:, :, q0:q0 + G, :],
                                          in_=st)

                qkv, xr = start_group(0)
                for g in range(NG):
                    ot_full = attn_phase(qkv, pools)
                    if g + 1 < NG:
                        nqkv, nxr = start_group(g + 1)
                    finish_group(g, ot_full, xr)
                    if g + 1 < NG:
                        qkv, xr = nqkv, nxr

        if timed:
            with tc.For_i(0, reps, 1):
                run_pass(True)
                run_pass(False)
            nc.sync.dma_start(out=tick, in_=ones_sb[0:1, :])
        else:
            run_pass(True)
            run_pass(False)

    nc.compile()
    return nc


def _get_program():
    global _PROG
    if _PROG is None:
        _PROG = _build_program()
    return _PROG


def _host_prep(xs, wmap):
    """Per-batch input maps for the SPMD run."""
    xs = np.asarray(xs, dtype=np.float32)
    base = {}
    for n, w in wmap.items():
        wt8 = np.ascontiguousarray(np.asarray(w, dtype=np.float32).T) * WS
        if n.startswith("wo") and not FP8_O:
            base[n] = (wt8 / WS).astype(_BF16)
        else:
            base[n + "8"] = wt8.astype(_F8)

    in_maps = []
    for b in range(NCORES):
        xb = np.ascontiguousarray(xs[b])                        # (C, H, W) f32
        xss = xb * XS
        xT = np.ascontiguousarray(np.swapaxes(xb, 1, 2))        # (C, W, H)
        m = dict(base)
        m["x8"] = xss.astype(_F8)
        m["xt8"] = np.ascontiguousarray(np.swapaxes(xss, 1, 2)).astype(_F8)
        m["xt16"] = xT.astype(np.float16)
        in_maps.append(m)
    return in_maps


def kernel(xs, Wq_h, Wk_h, Wv_h, Wo_h, Wq_w, Wk_w, Wv_w, Wo_w):
    from concourse.bass_utils import run_bass_kernel_spmd

    nc = _get_program()

    wmap = {
        "wq_w": Wq_w, "wk_w": Wk_w, "wv_w": Wv_w, "wo_w": Wo_w,
        "wq_h": Wq_h, "wk_h": Wk_h, "wv_h": Wv_h, "wo_h": Wo_h,
    }
    in_maps = _host_prep(xs, wmap)
    res = run_bass_kernel_spmd(nc, in_maps, core_ids=list(range(NCORES)))
    return np.stack(
        [res.results[b]["out16"].astype(np.float32) for b in range(NCORES)],
        axis=0)


# revision 18
# speedup vs baseline: 1.4296x; 1.0469x over previous
"""Axial attention block (B=8, C=512, H=W=128, 8 heads) on 8 Trainium2 cores.

Sharding: data-parallel over batch — one batch element per NeuronCore. Each
core runs both axial passes on its (C, H, W) slice and produces the full
residual sum xs + oh + ow.

v3 design (HBM-traffic-minimized, engine-balanced, PE-pipelined):
  - oh never round-trips to DRAM: pass 1 writes (oh + xs) into a
    SBUF-resident f16 buffer ohsb[co] [P, w, h]; pass 2 reads it back
    transposed as the residual for the final sum.  DRAM traffic per core:
    xt8 + x8 (fp8, 16.8 MB), xt16 (f16 transposed xs, 16.8 MB), out16
    (f16 output, 16.8 MB) ~50 MB vs 150 MB in v1 — the 8 cores contend
    for shared HBM, so traffic dominates at 8-core scale.
  - Softmax denominators in a parity-split PSUM layout: two ones-matmuls
    write denom rows into partitions 0:64 (even heads) / 64:128 (odd
    heads) of one [P,512] tile, so reciprocal and normalize run once at
    full width.
  - PSUM rings sized to exactly 8 banks: proj [P,512]x3, scores/denoms
    shared ring [P,512]x3 (st2a, st2b, r2 per seq), AV/O-proj shared ring
    [P,512]x2 (po per seq, pod per c_out).
  - exp split into parity halves so AV of even heads starts ~600 ns
    earlier; evacuations spread across ScalarE/VectorE/GpSimd so no
    elementwise engine exceeds TensorE busy time.
  - Software pipelining: group n+1's Q/K/V projections are emitted
    between group n's attention tail and its O-projection, hiding the
    exp/AV/normalize latency of the last sequences.
"""
import os
import numpy as np
import ml_dtypes

P = 128          # partitions
C = 512          # channels
S = 128          # sequence length (H and W)
NCB = C // P     # channel blocks
NH = 8           # heads
DH = C // NH     # head dim
G = 4            # sequences per group
NG = S // G      # groups per pass
NCORES = 8

_BF16 = ml_dtypes.bfloat16
_F8 = ml_dtypes.float8_e4m3

FP8_O = int(os.environ.get("K_FP8_O", "0"))  # fp8 DoubleRow O projection
FP8O_H = int(os.environ.get("K_FP8O_H", "1"))  # fp8 O-proj in height pass only
# timing-diagnosis hacks (produce WRONG results; never set when grading)
HACK_CONTIG = int(os.environ.get("K_HACK_CONTIG", "0"))
# ohsb layout: 0 = [P, w, h] (pass-1-natural, pass 2 reads strided),
#              1 = [P, h, w] (pass-2-natural, pass 1 writes strided).
# Strided DVE reads measured ~5x slower than contiguous on HW; strided
# writes are nearly free, so default to the pass-2-natural layout.
OH_HW = int(os.environ.get("K_OH_HW", "1"))
XS = float(os.environ.get("K_XS", "16"))   # host pre-scale on x (fp8)
WS = float(os.environ.get("K_WS", "64"))   # host pre-scale on weights (fp8)
OS = 8.0         # on-device pre-scale on ot (fp8 O-proj input)

_PROG = None  # cached compiled Bass program


def _build_program(reps=None):
    """reps=None: normal external-I/O program.  reps=R: timing variant —
    internal DRAM I/O (zero-initialized on device), kernel body repeated R
    times in a hardware For_i loop, tiny tick output."""
    from contextlib import ExitStack
    import concourse.tile as tile
    from concourse import bacc, mybir

    f32 = mybir.dt.float32
    bf = mybir.dt.bfloat16
    f16 = mybir.dt.float16
    f8 = mybir.dt.float8e4
    DRM = mybir.MatmulPerfMode.DoubleRow
    Exp = mybir.ActivationFunctionType.Exp
    MUL = mybir.AluOpType.mult
    ADD = mybir.AluOpType.add

    qk_sc = 1.0 / (XS * WS)
    vt_sc = 1.0 / (XS * WS)

    timed = reps is not None
    ext_in = {} if timed else {"kind": "ExternalInput"}

    nc = bacc.Bacc("TRN2", target_bir_lowering=False, debug=False)

    x8 = nc.dram_tensor("x8", [C, S, S], f8, **ext_in).ap()      # (C,H,W)
    xt8 = nc.dram_tensor("xt8", [C, S, S], f8, **ext_in).ap()    # (C,W,H)
    xt16 = nc.dram_tensor("xt16", [C, S, S], f16, **ext_in).ap() # (C,W,H)
    qkv_names = ["wq_w", "wk_w", "wv_w", "wq_h", "wk_h", "wv_h"]
    o_names = ["wo_w", "wo_h"]
    if FP8_O:
        f8_names = qkv_names + o_names
        bf_names = []
    elif FP8O_H:
        f8_names = qkv_names + ["wo_h"]
        bf_names = ["wo_w"]
    else:
        f8_names = qkv_names
        bf_names = o_names
    w8t = {n: nc.dram_tensor(n + "8", [C, C], f8, **ext_in).ap()
           for n in f8_names}
    wt = {n: nc.dram_tensor(n, [C, C], bf, **ext_in).ap() for n in bf_names}
    if timed:
        out16 = nc.dram_tensor("out16", [C, S, S], f16).ap()
    else:
        out16 = nc.dram_tensor("out16", [C, S, S], f16,
                               kind="ExternalOutput").ap()
    tick = nc.dram_tensor("tick", [1, P], bf, kind="ExternalOutput").ap() if timed else None

    # transposed DRAM views: partition = channel-within-block
    xt8v = xt8.rearrange("(a j k) w h -> a k j w h", a=2, j=2)
    x8v = x8.rearrange("(a j k) h w -> a k j h w", a=2, j=2)
    xt16v = xt16.rearrange("(c k) w h -> k c w h", c=NCB)
    out16v = out16.rearrange("(c k) h w -> k c h w", c=NCB)

    with tile.TileContext(nc) as tc, ExitStack() as topctx:
        const = topctx.enter_context(tc.tile_pool(name="const", bufs=1))

        # weights resident in SBUF
        w8_sb = {}   # name -> [2 pair tiles [P, 2, C] fp8]
        wb_sb = {}   # name -> [4 ci tiles [P, C] bf16]
        for n, t in w8t.items():
            tiles = []
            src = t.rearrange("(a j k) n -> a k j n", a=2, j=2)
            for pair in range(2):
                tl = const.tile([P, 2, C], f8, tag=f"w8_{n}_{pair}",
                                name=f"w8_{n}_{pair}")
                nc.sync.dma_start(out=tl, in_=src[pair])
                tiles.append(tl)
            w8_sb[n] = tiles
        for n, t in wt.items():
            tiles = []
            for ci in range(NCB):
                tl = const.tile([P, C], bf, tag=f"w_{n}_{ci}", name=f"w_{n}_{ci}")
                nc.sync.dma_start(out=tl, in_=t[ci * P:(ci + 1) * P, :])
                tiles.append(tl)
            wb_sb[n] = tiles
        ones_sb = const.tile([P, P], bf, tag="ones", name="ones")
        nc.vector.memset(ones_sb, 1.0)

        # SBUF-resident oh+xs accumulator, f16, one tile per c_out block
        ohsb = []
        for co in range(NCB):
            t = const.tile([P, S, S], f16, tag=f"ohsb{co}", name=f"ohsb{co}")
            ohsb.append(t)

        if timed:
            zb = const.tile([P, 1024], bf, tag="zb", name="zb")
            nc.vector.memset(zb, 0.0)
            z16 = const.tile([P, 1024], f16, tag="z16", name="z16")
            nc.vector.memset(z16, 0.0)
            z8 = const.tile([P, 1024], f8, tag="z8", name="z8")
            nc.vector.memset(z8, 0.0)
            for cb in range(NCB):
                cs = slice(cb * P, (cb + 1) * P)
                for j in range(16):
                    js = slice(j * 8, (j + 1) * 8)
                    nc.sync.dma_start(
                        out=x8[cs, js, :],
                        in_=z8.rearrange("p (a b) -> p a b", a=8))
                    nc.sync.dma_start(
                        out=xt8[cs, js, :],
                        in_=z8.rearrange("p (a b) -> p a b", a=8))
                    nc.sync.dma_start(
                        out=xt16[cs, js, :],
                        in_=z16.rearrange("p (a b) -> p a b", a=8))
                for n in w8t:
                    nc.sync.dma_start(out=w8t[n][cs, :], in_=z8[:, 0:C])
                for n in wt:
                    nc.sync.dma_start(out=wt[n][cs, :], in_=zb[:, 0:C])

        # elementwise-engine dispatch helpers -----------------------------
        def ew_scale(eng, out_ap, in_ap, sc):
            if eng == "act":
                nc.scalar.mul(out_ap, in_ap, sc) if sc != 1.0 else nc.scalar.copy(out_ap, in_ap)
            elif eng == "dve":
                if sc != 1.0:
                    nc.vector.tensor_scalar_mul(out_ap, in_ap, sc)
                else:
                    nc.vector.tensor_copy(out_ap, in_ap)
            else:
                if sc != 1.0:
                    nc.gpsimd.tensor_scalar_mul(out_ap, in_ap, sc)
                else:
                    nc.gpsimd.tensor_copy(out_ap, in_ap)

        def ew_stage(eng, out_ap, in0_ap, in1_ap, stg_sc):
            """out = in0 * stg_sc + in1 on a chosen engine."""
            mod = {"dve": nc.vector, "pool": nc.gpsimd}[eng]
            if stg_sc != 1.0:
                mod.scalar_tensor_tensor(out=out_ap, in0=in0_ap, scalar=stg_sc,
                                         in1=in1_ap, op0=MUL, op1=ADD)
            else:
                mod.tensor_add(out_ap, in0_ap, in1_ap)

        # GpSimd/Pool has no PSUM port, so every PSUM-reading op must run on
        # ScalarE (act) or VectorE (dve).  tensor_tensor ops (stage adds,
        # normalize) are DVE-only; exp is Act-only; the plain evacuations
        # are split to balance the two queues.
        QK_ENG = ["act"] * 8
        VT_ENG = ["act", "dve", "dve", "dve"]
        STG_ENG = ["dve", "dve", "dve", "dve"]

        def recip(rr, r2):
            nc.vector.reciprocal_approx_fast(out=rr, in_=r2)

        def proj_phase(src_t, wq, wk, wv, pools):
            """Q, K, V projections (fp8 DoubleRow) for one group."""
            (qk_pool, vt_pool, ot_pool, et_pool, rr_pool,
             proj_ps, sr_ps, po_ps) = pools
            qt_sb, kt_sb = [], []
            for wmat, dst_list, nm in ((wq, qt_sb, "qt"), (wk, kt_sb, "kt")):
                for co in range(NCB):
                    pp = proj_ps.tile([P, G * S], f32, tag="proj", name="pp")
                    for pair in range(2):
                        nc.tensor.matmul(
                            pp,
                            lhsT=wmat[pair][:, :, co * P:(co + 1) * P],
                            rhs=src_t[pair],
                            start=(pair == 0), stop=(pair == 1),
                            perf_mode=DRM)
                    sb_t = qk_pool.tile([P, G * S], bf, tag=f"{nm}{co}",
                                        name=f"{nm}{co}")
                    ew_scale(QK_ENG[(0 if nm == "qt" else NCB) + co],
                             sb_t, pp, qk_sc)
                    dst_list.append(sb_t)
            vt_sb = []
            for sq in range(G):
                pv = proj_ps.tile([P, C], f32, tag="proj", name="pv")
                for pair in range(2):
                    nc.tensor.matmul(
                        pv,
                        lhsT=src_t[pair][:, :, sq, :],
                        rhs=wv[pair],
                        start=(pair == 0), stop=(pair == 1),
                        perf_mode=DRM)
                vt = vt_pool.tile([P, C], bf, tag=f"vt{sq}", name=f"vt{sq}")
                ew_scale(VT_ENG[sq], vt, pv, vt_sc)
                vt_sb.append(vt)
            return qt_sb, kt_sb, vt_sb

        def attn_phase(qkv, pools, fp8o):
            """Scores, softmax, AV for all G sequences -> normalized ot tile."""
            qt_sb, kt_sb, vt_sb = qkv
            (qk_pool, vt_pool, ot_pool, et_pool, rr_pool,
             proj_ps, sr_ps, po_ps) = pools
            ot_full = ot_pool.tile([P, NCB, G * S], f8 if fp8o else bf,
                                   tag="ot", name="ot")
            for sq in range(G):
                ssl = slice(sq * S, (sq + 1) * S)
                et = et_pool.tile([P, 1024], bf, tag="et", name="et")
                # even heads h=2cb: rows 0:64 of block cb; odd: rows 64:128
                st2a = sr_ps.tile([P, 512], f32, tag="sr", name="st2a")
                for cb in range(NCB):
                    nc.tensor.matmul(
                        st2a[:, cb * S:(cb + 1) * S],
                        lhsT=kt_sb[cb][0:DH, ssl],
                        rhs=qt_sb[cb][0:DH, ssl],
                        start=True, stop=True)
                nc.scalar.activation(out=et[:, 0:512], in_=st2a, func=Exp,
                                     scale=DH ** -0.5)
                st2b = sr_ps.tile([P, 512], f32, tag="sr", name="st2b")
                for cb in range(NCB):
                    nc.tensor.matmul(
                        st2b[:, cb * S:(cb + 1) * S],
                        lhsT=kt_sb[cb][DH:P, ssl],
                        rhs=qt_sb[cb][DH:P, ssl],
                        start=True, stop=True)
                nc.scalar.activation(out=et[:, 512:1024], in_=st2b, func=Exp,
                                     scale=DH ** -0.5)
                # denominators, parity-split over partition halves
                r2 = sr_ps.tile([P, 512], f32, tag="sr", name="r2")
                nc.tensor.matmul(r2[0:DH, :], lhsT=ones_sb[:, 0:DH],
                                 rhs=et[:, 0:512], start=True, stop=True)
                nc.tensor.matmul(r2[DH:P, :], lhsT=ones_sb[:, 0:DH],
                                 rhs=et[:, 512:1024], start=True, stop=True)
                rr = rr_pool.tile([P, 512], f32, tag="rr", name="rr")
                recip(rr, r2)
                # AV on unnormalized exp; normalize fuses into the
                # psum->sbuf evacuation (recip overlaps AV on TensorE)
                po = po_ps.tile([P, 512], f32, tag="po", name="po")
                for h in range(NH):
                    par, cb = h % 2, h // 2
                    nc.tensor.matmul(
                        po[par * DH:(par + 1) * DH, cb * S:(cb + 1) * S],
                        lhsT=vt_sb[sq][:, h * DH:(h + 1) * DH],
                        rhs=et[:, par * 512 + cb * S:par * 512 + (cb + 1) * S],
                        start=True, stop=True)
                # po row par*64+d, col block cb holds head h=2cb+par ->
                # normalizer rr[p, cb*128+s] has matching parity by p-half
                if fp8o:
                    nc.vector.scalar_tensor_tensor(
                        out=ot_full[:, :, ssl],
                        in0=po.rearrange("p (c s) -> p c s", c=NCB),
                        scalar=OS,
                        in1=rr.rearrange("p (c s) -> p c s", c=NCB),
                        op0=MUL, op1=MUL)
                else:
                    nc.vector.tensor_mul(
                        ot_full[:, :, ssl],
                        po.rearrange("p (c s) -> p c s", c=NCB),
                        rr.rearrange("p (c s) -> p c s", c=NCB))
            return ot_full

        def oproj_phase(ot_full, wo, pools, fp8o):
            """O-projection -> 4 psum tiles [P, G*S], one per c_out block."""
            (qk_pool, vt_pool, ot_pool, et_pool, rr_pool,
             proj_ps, sr_ps, po_ps) = pools
            pods = []
            if fp8o:
                otv = ot_full.rearrange("p (a j) s -> p a j s", a=2)
            for co in range(NCB):
                pod = po_ps.tile([P, 512], f32, tag="po", name="pod")
                if fp8o:
                    for pair in range(2):
                        nc.tensor.matmul(
                            pod,
                            lhsT=wo[pair][:, :, co * P:(co + 1) * P],
                            rhs=otv[:, pair],
                            start=(pair == 0), stop=(pair == 1),
                            perf_mode=DRM)
                else:
                    for ci in range(NCB):
                        nc.tensor.matmul(
                            pod,
                            lhsT=wo[ci][:, co * P:(co + 1) * P],
                            rhs=ot_full[:, ci, :],
                            start=(ci == 0), stop=(ci == NCB - 1))
                pods.append(pod)
            return pods

        def get_w(kind, suffix, fp8o=False):
            n = f"w{kind}_{suffix}"
            if kind in ("q", "k", "v"):
                return w8_sb[n]
            return w8_sb[n] if fp8o else wb_sb[n]

        def mk_pools(ctx, sfx):
            qk_pool = ctx.enter_context(tc.tile_pool(name=f"qk{sfx}", bufs=2))
            vt_pool = ctx.enter_context(tc.tile_pool(name=f"vt{sfx}", bufs=2))
            ot_pool = ctx.enter_context(tc.tile_pool(name=f"ot{sfx}", bufs=2))
            et_pool = ctx.enter_context(tc.tile_pool(name=f"et{sfx}", bufs=2))
            rr_pool = ctx.enter_context(tc.tile_pool(name=f"rr{sfx}", bufs=2))
            proj_ps = ctx.enter_context(
                tc.tile_pool(name=f"pps{sfx}", bufs=3, space="PSUM"))
            sr_ps = ctx.enter_context(
                tc.tile_pool(name=f"srs{sfx}", bufs=3, space="PSUM"))
            po_ps = ctx.enter_context(
                tc.tile_pool(name=f"pos{sfx}", bufs=2, space="PSUM"))
            return (qk_pool, vt_pool, ot_pool, et_pool, rr_pool,
                    proj_ps, sr_ps, po_ps)

        def load_src(src_pool, xsrcv, q0):
            src_t = []
            for pair in range(2):
                t = src_pool.tile([P, 2, G, S], f8, tag=f"src{pair}",
                                  name=f"src{pair}")
                nc.sync.dma_start(out=t, in_=xsrcv[pair, :, :, q0:q0 + G, :])
                src_t.append(t)
            return src_t

        def run_pass(is_height):
            """Software-pipelined group loop for one axial pass."""
            sfx = "1" if is_height else "2"
            suffix = "h" if is_height else "w"
            fp8o = bool(FP8_O) or (bool(FP8O_H) and is_height)
            stg_sc = 1.0 / (WS * OS) if fp8o else 1.0
            wq, wk, wv, wo = (get_w("q", suffix), get_w("k", suffix),
                              get_w("v", suffix), get_w("o", suffix, fp8o))
            with ExitStack() as ctx:
                src_pool = ctx.enter_context(
                    tc.tile_pool(name=f"src{sfx}", bufs=2))
                if is_height:
                    xr_pool = ctx.enter_context(
                        tc.tile_pool(name="xr1", bufs=2))
                else:
                    stage_pool = ctx.enter_context(
                        tc.tile_pool(name="stg2", bufs=2))
                pools = mk_pools(ctx, sfx)

                def start_group(g):
                    q0 = g * G
                    src_t = load_src(src_pool, xt8v if is_height else x8v, q0)
                    xr = None
                    if is_height:
                        xr = xr_pool.tile([P, NCB, G, S], f16, tag="xr",
                                          name="xr")
                        nc.sync.dma_start(out=xr,
                                          in_=xt16v[:, :, q0:q0 + G, :])
                    qkv = proj_phase(src_t, wq, wk, wv, pools)
                    return qkv, xr

                def finish_group(g, ot_full, xr):
                    q0 = g * G
                    pods = oproj_phase(ot_full, wo, pools, fp8o)
                    if is_height:
                        for co in range(NCB):
                            if OH_HW:
                                out_ap = ohsb[co][:, :, q0:q0 + G].rearrange(
                                    "p h q -> p q h")
                            else:
                                out_ap = ohsb[co][:, q0:q0 + G, :]
                            ew_stage(
                                STG_ENG[co],
                                out_ap,
                                pods[co].rearrange("p (q s) -> p q s", q=G),
                                xr[:, co], stg_sc)
                    else:
                        st = stage_pool.tile([P, NCB, G, S], f16, tag="st",
                                             name="st")
                        for co in range(NCB):
                            if OH_HW:
                                resid_ap = ohsb[co][:, q0:q0 + G, :]
                            elif HACK_CONTIG:
                                resid_ap = ohsb[co][:, q0:q0 + G, :]
                            else:
                                resid_ap = ohsb[co][:, :, q0:q0 + G].rearrange(
                                    "p w i -> p i w")
                            ew_stage(
                                STG_ENG[co],
                                st[:, co],
                                pods[co].rearrange("p (q s) -> p q s", q=G),
                                resid_ap, stg_sc)
                        nc.sync.dma_start(out=out16v[:, :, q0:q0 + G, :],
                                          in_=st)

                qkv, xr = start_group(0)
                for g in range(NG):
                    ot_full = attn_phase(qkv, pools, fp8o)
                    if g + 1 < NG:
                        nqkv, nxr = start_group(g + 1)
                    finish_group(g, ot_full, xr)
                    if g + 1 < NG:
                        qkv, xr = nqkv, nxr

        if timed:
            with tc.For_i(0, reps, 1):
                run_pass(True)
                run_pass(False)
            nc.sync.dma_start(out=tick, in_=ones_sb[0:1, :])
        else:
            run_pass(True)
            run_pass(False)

    nc.compile()
    return nc


def _get_program():
    global _PROG
    if _PROG is None:
        _PROG = _build_program()
    return _PROG


def _host_prep(xs, wmap):
    """Per-batch input maps for the SPMD run."""
    xs = np.asarray(xs, dtype=np.float32)
    base = {}
    for n, w in wmap.items():
        wt8 = np.ascontiguousarray(np.asarray(w, dtype=np.float32).T) * WS
        if n.startswith("wo") and not (FP8_O or FP8O_H):
            base[n] = (wt8 / WS).astype(_BF16)
        elif n.startswith("wo") and FP8O_H and not FP8_O:
            # height-pass O fp8, width-pass O bf16: ship both forms
            if n == "wo_h":
                base[n + "8"] = wt8.astype(_F8)
            else:
                base[n] = (wt8 / WS).astype(_BF16)
        else:
            base[n + "8"] = wt8.astype(_F8)

    in_maps = []
    for b in range(NCORES):
        xb = np.ascontiguousarray(xs[b])                        # (C, H, W) f32
        xss = xb * XS
        xT = np.ascontiguousarray(np.swapaxes(xb, 1, 2))        # (C, W, H)
        m = dict(base)
        m["x8"] = xss.astype(_F8)
        m["xt8"] = np.ascontiguousarray(np.swapaxes(xss, 1, 2)).astype(_F8)
        m["xt16"] = xT.astype(np.float16)
        in_maps.append(m)
    return in_maps


def kernel(xs, Wq_h, Wk_h, Wv_h, Wo_h, Wq_w, Wk_w, Wv_w, Wo_w):
    from concourse.bass_utils import run_bass_kernel_spmd

    nc = _get_program()

    wmap = {
        "wq_w": Wq_w, "wk_w": Wk_w, "wv_w": Wv_w, "wo_w": Wo_w,
        "wq_h": Wq_h, "wk_h": Wk_h, "wv_h": Wv_h, "wo_h": Wo_h,
    }
    in_maps = _host_prep(xs, wmap)
    res = run_bass_kernel_spmd(nc, in_maps, core_ids=list(range(NCORES)))
    return np.stack(
        [res.results[b]["out16"].astype(np.float32) for b in range(NCORES)],
        axis=0)


# revision 26
# speedup vs baseline: 1.5110x; 1.0569x over previous
"""Axial attention block (B=8, C=512, H=W=128, 8 heads) on 8 Trainium2 cores.

Sharding: data-parallel over batch — one batch element per NeuronCore. Each
core runs both axial passes on its (C, H, W) slice and produces the full
residual sum xs + oh + ow.

v3 design (HBM-traffic-minimized, engine-balanced, PE-pipelined):
  - oh never round-trips to DRAM: pass 1 writes (oh + xs) into a
    SBUF-resident f16 buffer ohsb[co] [P, w, h]; pass 2 reads it back
    transposed as the residual for the final sum.  DRAM traffic per core:
    xt8 + x8 (fp8, 16.8 MB), xt16 (f16 transposed xs, 16.8 MB), out16
    (f16 output, 16.8 MB) ~50 MB vs 150 MB in v1 — the 8 cores contend
    for shared HBM, so traffic dominates at 8-core scale.
  - Softmax denominators in a parity-split PSUM layout: two ones-matmuls
    write denom rows into partitions 0:64 (even heads) / 64:128 (odd
    heads) of one [P,512] tile, so reciprocal and normalize run once at
    full width.
  - PSUM rings sized to exactly 8 banks: proj [P,512]x3, scores/denoms
    shared ring [P,512]x3 (st2a, st2b, r2 per seq), AV/O-proj shared ring
    [P,512]x2 (po per seq, pod per c_out).
  - exp split into parity halves so AV of even heads starts ~600 ns
    earlier; evacuations spread across ScalarE/VectorE/GpSimd so no
    elementwise engine exceeds TensorE busy time.
  - Software pipelining: group n+1's Q/K/V projections are emitted
    between group n's attention tail and its O-projection, hiding the
    exp/AV/normalize latency of the last sequences.
"""
import os
import numpy as np
import ml_dtypes

P = 128          # partitions
C = 512          # channels
S = 128          # sequence length (H and W)
NCB = C // P     # channel blocks
NH = 8           # heads
DH = C // NH     # head dim
G = 4            # sequences per group
NG = S // G      # groups per pass
NCORES = 8

_BF16 = ml_dtypes.bfloat16
_F8 = ml_dtypes.float8_e4m3

FP8_O = int(os.environ.get("K_FP8_O", "0"))  # fp8 DoubleRow O projection
FP8O_H = int(os.environ.get("K_FP8O_H", "1"))  # fp8 O-proj in height pass only
# timing-diagnosis hacks (produce WRONG results; never set when grading)
HACK_CONTIG = int(os.environ.get("K_HACK_CONTIG", "0"))
# ohsb layout: 0 = [P, w, h] (pass-1-natural, pass 2 reads strided),
#              1 = [P, h, w] (pass-2-natural, pass 1 writes strided).
# Strided DVE reads measured ~5x slower than contiguous on HW; strided
# writes are nearly free, so default to the pass-2-natural layout.
OH_HW = int(os.environ.get("K_OH_HW", "1"))
XS = float(os.environ.get("K_XS", "16"))   # host pre-scale on x (fp8)
WS = float(os.environ.get("K_WS", "64"))   # host pre-scale on weights (fp8)
OS = 8.0         # on-device pre-scale on ot (fp8 O-proj input)

_PROG = None  # cached compiled Bass program


def _build_program(reps=None):
    """reps=None: normal external-I/O program.  reps=R: timing variant —
    internal DRAM I/O (zero-initialized on device), kernel body repeated R
    times in a hardware For_i loop, tiny tick output."""
    from contextlib import ExitStack
    import concourse.tile as tile
    from concourse import bacc, mybir

    f32 = mybir.dt.float32
    bf = mybir.dt.bfloat16
    f16 = mybir.dt.float16
    f8 = mybir.dt.float8e4
    DRM = mybir.MatmulPerfMode.DoubleRow
    Exp = mybir.ActivationFunctionType.Exp
    MUL = mybir.AluOpType.mult
    ADD = mybir.AluOpType.add

    qk_sc = 1.0 / (XS * WS)
    vt_sc = 1.0 / (XS * WS)

    timed = reps is not None
    ext_in = {} if timed else {"kind": "ExternalInput"}

    nc = bacc.Bacc("TRN2", target_bir_lowering=False, debug=False)

    x8 = nc.dram_tensor("x8", [C, S, S], f8, **ext_in).ap()      # (C,H,W)
    xt8 = nc.dram_tensor("xt8", [C, S, S], f8, **ext_in).ap()    # (C,W,H)
    xt16 = nc.dram_tensor("xt16", [C, S, S], f16, **ext_in).ap() # (C,W,H)
    qkv_names = ["wq_w", "wk_w", "wv_w", "wq_h", "wk_h", "wv_h"]
    o_names = ["wo_w", "wo_h"]
    if FP8_O:
        f8_names = qkv_names + o_names
        bf_names = []
    elif FP8O_H:
        f8_names = qkv_names + ["wo_h"]
        bf_names = ["wo_w"]
    else:
        f8_names = qkv_names
        bf_names = o_names
    w8t = {n: nc.dram_tensor(n + "8", [C, C], f8, **ext_in).ap()
           for n in f8_names}
    wt = {n: nc.dram_tensor(n, [C, C], bf, **ext_in).ap() for n in bf_names}
    if timed:
        out16 = nc.dram_tensor("out16", [C, S, S], f16).ap()
    else:
        out16 = nc.dram_tensor("out16", [C, S, S], f16,
                               kind="ExternalOutput").ap()
    tick = nc.dram_tensor("tick", [1, P], bf, kind="ExternalOutput").ap() if timed else None

    # transposed DRAM views: partition = channel-within-block
    xt8v = xt8.rearrange("(a j k) w h -> a k j w h", a=2, j=2)
    x8v = x8.rearrange("(a j k) h w -> a k j h w", a=2, j=2)
    xt16v = xt16.rearrange("(c k) w h -> k c w h", c=NCB)
    out16v = out16.rearrange("(c k) h w -> k c h w", c=NCB)

    with tile.TileContext(nc) as tc, ExitStack() as topctx:
        const = topctx.enter_context(tc.tile_pool(name="const", bufs=1))

        # weights resident in SBUF
        w8_sb = {}   # name -> [2 pair tiles [P, 2, C] fp8]
        wb_sb = {}   # name -> [4 ci tiles [P, C] bf16]
        for n, t in w8t.items():
            tiles = []
            src = t.rearrange("(a j k) n -> a k j n", a=2, j=2)
            for pair in range(2):
                tl = const.tile([P, 2, C], f8, tag=f"w8_{n}_{pair}",
                                name=f"w8_{n}_{pair}")
                nc.sync.dma_start(out=tl, in_=src[pair])
                tiles.append(tl)
            w8_sb[n] = tiles
        for n, t in wt.items():
            tiles = []
            for ci in range(NCB):
                tl = const.tile([P, C], bf, tag=f"w_{n}_{ci}", name=f"w_{n}_{ci}")
                nc.sync.dma_start(out=tl, in_=t[ci * P:(ci + 1) * P, :])
                tiles.append(tl)
            wb_sb[n] = tiles
        ones_sb = const.tile([P, P], bf, tag="ones", name="ones")
        nc.vector.memset(ones_sb, 1.0)

        # SBUF-resident oh+xs accumulator, f16, one tile per c_out block
        ohsb = []
        for co in range(NCB):
            t = const.tile([P, S, S], f16, tag=f"ohsb{co}", name=f"ohsb{co}")
            ohsb.append(t)

        if timed:
            zb = const.tile([P, 1024], bf, tag="zb", name="zb")
            nc.vector.memset(zb, 0.0)
            z16 = const.tile([P, 1024], f16, tag="z16", name="z16")
            nc.vector.memset(z16, 0.0)
            z8 = const.tile([P, 1024], f8, tag="z8", name="z8")
            nc.vector.memset(z8, 0.0)
            for cb in range(NCB):
                cs = slice(cb * P, (cb + 1) * P)
                for j in range(16):
                    js = slice(j * 8, (j + 1) * 8)
                    nc.sync.dma_start(
                        out=x8[cs, js, :],
                        in_=z8.rearrange("p (a b) -> p a b", a=8))
                    nc.sync.dma_start(
                        out=xt8[cs, js, :],
                        in_=z8.rearrange("p (a b) -> p a b", a=8))
                    nc.sync.dma_start(
                        out=xt16[cs, js, :],
                        in_=z16.rearrange("p (a b) -> p a b", a=8))
                for n in w8t:
                    nc.sync.dma_start(out=w8t[n][cs, :], in_=z8[:, 0:C])
                for n in wt:
                    nc.sync.dma_start(out=wt[n][cs, :], in_=zb[:, 0:C])

        # elementwise-engine dispatch helpers -----------------------------
        def ew_scale(eng, out_ap, in_ap, sc):
            if eng == "act":
                nc.scalar.mul(out_ap, in_ap, sc) if sc != 1.0 else nc.scalar.copy(out_ap, in_ap)
            elif eng == "dve":
                if sc != 1.0:
                    nc.vector.tensor_scalar_mul(out_ap, in_ap, sc)
                else:
                    nc.vector.tensor_copy(out_ap, in_ap)
            else:
                if sc != 1.0:
                    nc.gpsimd.tensor_scalar_mul(out_ap, in_ap, sc)
                else:
                    nc.gpsimd.tensor_copy(out_ap, in_ap)

        def ew_stage(eng, out_ap, in0_ap, in1_ap, stg_sc):
            """out = in0 * stg_sc + in1 on a chosen engine."""
            mod = {"dve": nc.vector, "pool": nc.gpsimd}[eng]
            if stg_sc != 1.0:
                mod.scalar_tensor_tensor(out=out_ap, in0=in0_ap, scalar=stg_sc,
                                         in1=in1_ap, op0=MUL, op1=ADD)
            else:
                mod.tensor_add(out_ap, in0_ap, in1_ap)

        # GpSimd/Pool has no PSUM port, so every PSUM-reading op must run on
        # ScalarE (act) or VectorE (dve).  tensor_tensor ops (stage adds,
        # normalize) are DVE-only; exp is Act-only; the plain evacuations
        # are split to balance the two queues.
        QK_ENG = ["act"] * 8
        VT_ENG = ["act", "dve", "dve", "dve"]
        STG_ENG = ["dve", "dve", "dve", "dve"]

        def recip(rr, r2):
            nc.vector.reciprocal_approx_fast(out=rr, in_=r2)

        def proj_phase(src_t, wq, wk, wv, pools):
            """Q, K, V projections (fp8 DoubleRow) for one group."""
            (qk_pool, vt_pool, ot_pool, et_pool, rr_pool,
             proj_ps, sr_ps, po_ps) = pools
            qt_sb, kt_sb = [], []
            for wmat, dst_list, nm in ((wq, qt_sb, "qt"), (wk, kt_sb, "kt")):
                for co in range(NCB):
                    pp = proj_ps.tile([P, G * S], f32, tag="proj", name="pp")
                    for pair in range(2):
                        nc.tensor.matmul(
                            pp,
                            lhsT=wmat[pair][:, :, co * P:(co + 1) * P],
                            rhs=src_t[pair],
                            start=(pair == 0), stop=(pair == 1),
                            perf_mode=DRM)
                    sb_t = qk_pool.tile([P, G * S], bf, tag=f"{nm}{co}",
                                        name=f"{nm}{co}")
                    ew_scale(QK_ENG[(0 if nm == "qt" else NCB) + co],
                             sb_t, pp, qk_sc)
                    dst_list.append(sb_t)
            vt_sb = []
            for sq in range(G):
                pv = proj_ps.tile([P, C], f32, tag="proj", name="pv")
                for pair in range(2):
                    nc.tensor.matmul(
                        pv,
                        lhsT=src_t[pair][:, :, sq, :],
                        rhs=wv[pair],
                        start=(pair == 0), stop=(pair == 1),
                        perf_mode=DRM)
                vt = vt_pool.tile([P, C], bf, tag=f"vt{sq}", name=f"vt{sq}")
                ew_scale(VT_ENG[sq], vt, pv, vt_sc)
                vt_sb.append(vt)
            return qt_sb, kt_sb, vt_sb

        def attn_phase(qkv, pools, fp8o):
            """Scores, softmax, AV for all G sequences -> normalized ot tile."""
            qt_sb, kt_sb, vt_sb = qkv
            (qk_pool, vt_pool, ot_pool, et_pool, rr_pool,
             proj_ps, sr_ps, po_ps) = pools
            ot_full = ot_pool.tile([P, NCB, G * S], f8 if fp8o else bf,
                                   tag="ot", name="ot")
            for sq in range(G):
                ssl = slice(sq * S, (sq + 1) * S)
                et = et_pool.tile([P, 1024], bf, tag="et", name="et")
                # even heads h=2cb: rows 0:64 of block cb; odd: rows 64:128.
                # Interleave parities so consecutive matmuls sit in different
                # PE row-tiles (weight load overlaps the other tile's exec).
                st2a = sr_ps.tile([P, 512], f32, tag="sr", name="st2a")
                st2b = sr_ps.tile([P, 512], f32, tag="sr", name="st2b")
                for cb in range(NCB):
                    nc.tensor.matmul(
                        st2a[:, cb * S:(cb + 1) * S],
                        lhsT=kt_sb[cb][0:DH, ssl],
                        rhs=qt_sb[cb][0:DH, ssl],
                        start=True, stop=True)
                    nc.tensor.matmul(
                        st2b[:, cb * S:(cb + 1) * S],
                        lhsT=kt_sb[cb][DH:P, ssl],
                        rhs=qt_sb[cb][DH:P, ssl],
                        start=True, stop=True)
                nc.scalar.activation(out=et[:, 0:512], in_=st2a, func=Exp,
                                     scale=DH ** -0.5)
                nc.scalar.activation(out=et[:, 512:1024], in_=st2b, func=Exp,
                                     scale=DH ** -0.5)
                # denominators, parity-split over partition halves
                r2 = sr_ps.tile([P, 512], f32, tag="sr", name="r2")
                nc.tensor.matmul(r2[0:DH, :], lhsT=ones_sb[:, 0:DH],
                                 rhs=et[:, 0:512], start=True, stop=True)
                nc.tensor.matmul(r2[DH:P, :], lhsT=ones_sb[:, 0:DH],
                                 rhs=et[:, 512:1024], start=True, stop=True)
                rr = rr_pool.tile([P, 512], f32, tag="rr", name="rr")
                recip(rr, r2)
                # AV on unnormalized exp; normalize fuses into the
                # psum->sbuf evacuation (recip overlaps AV on TensorE)
                po = po_ps.tile([P, 512], f32, tag="po", name="po")
                for h in range(NH):
                    par, cb = h % 2, h // 2
                    nc.tensor.matmul(
                        po[par * DH:(par + 1) * DH, cb * S:(cb + 1) * S],
                        lhsT=vt_sb[sq][:, h * DH:(h + 1) * DH],
                        rhs=et[:, par * 512 + cb * S:par * 512 + (cb + 1) * S],
                        start=True, stop=True)
                # po row par*64+d, col block cb holds head h=2cb+par ->
                # normalizer rr[p, cb*128+s] has matching parity by p-half
                if fp8o:
                    nc.vector.scalar_tensor_tensor(
                        out=ot_full[:, :, ssl],
                        in0=po.rearrange("p (c s) -> p c s", c=NCB),
                        scalar=OS,
                        in1=rr.rearrange("p (c s) -> p c s", c=NCB),
                        op0=MUL, op1=MUL)
                else:
                    nc.vector.tensor_mul(
                        ot_full[:, :, ssl],
                        po.rearrange("p (c s) -> p c s", c=NCB),
                        rr.rearrange("p (c s) -> p c s", c=NCB))
            return ot_full

        def oproj_phase(ot_full, wo, pools, fp8o):
            """O-projection -> 4 psum tiles [P, G*S], one per c_out block."""
            (qk_pool, vt_pool, ot_pool, et_pool, rr_pool,
             proj_ps, sr_ps, po_ps) = pools
            pods = []
            if fp8o:
                otv = ot_full.rearrange("p (a j) s -> p a j s", a=2)
            for co in range(NCB):
                pod = po_ps.tile([P, 512], f32, tag="po", name="pod")
                if fp8o:
                    for pair in range(2):
                        nc.tensor.matmul(
                            pod,
                            lhsT=wo[pair][:, :, co * P:(co + 1) * P],
                            rhs=otv[:, pair],
                            start=(pair == 0), stop=(pair == 1),
                            perf_mode=DRM)
                else:
                    for ci in range(NCB):
                        nc.tensor.matmul(
                            pod,
                            lhsT=wo[ci][:, co * P:(co + 1) * P],
                            rhs=ot_full[:, ci, :],
                            start=(ci == 0), stop=(ci == NCB - 1))
                pods.append(pod)
            return pods

        def get_w(kind, suffix, fp8o=False):
            n = f"w{kind}_{suffix}"
            if kind in ("q", "k", "v"):
                return w8_sb[n]
            return w8_sb[n] if fp8o else wb_sb[n]

        def mk_pools(ctx, sfx):
            qk_pool = ctx.enter_context(tc.tile_pool(name=f"qk{sfx}", bufs=2))
            vt_pool = ctx.enter_context(tc.tile_pool(name=f"vt{sfx}", bufs=2))
            ot_pool = ctx.enter_context(tc.tile_pool(name=f"ot{sfx}", bufs=2))
            et_pool = ctx.enter_context(tc.tile_pool(name=f"et{sfx}", bufs=2))
            rr_pool = ctx.enter_context(tc.tile_pool(name=f"rr{sfx}", bufs=2))
            proj_ps = ctx.enter_context(
                tc.tile_pool(name=f"pps{sfx}", bufs=2, space="PSUM"))
            sr_ps = ctx.enter_context(
                tc.tile_pool(name=f"srs{sfx}", bufs=3, space="PSUM"))
            po_ps = ctx.enter_context(
                tc.tile_pool(name=f"pos{sfx}", bufs=3, space="PSUM"))
            return (qk_pool, vt_pool, ot_pool, et_pool, rr_pool,
                    proj_ps, sr_ps, po_ps)

        def load_src(src_pool, xsrcv, q0):
            src_t = []
            for pair in range(2):
                t = src_pool.tile([P, 2, G, S], f8, tag=f"src{pair}",
                                  name=f"src{pair}")
                nc.sync.dma_start(out=t, in_=xsrcv[pair, :, :, q0:q0 + G, :])
                src_t.append(t)
            return src_t

        def run_pass(is_height, src_pool, xst_pool, pools):
            """Software-pipelined group loop for one axial pass.  Pools are
            shared across both passes (and loop iterations) so no drain
            barriers appear inside the timed loop body."""
            suffix = "h" if is_height else "w"
            fp8o = bool(FP8_O) or (bool(FP8O_H) and is_height)
            stg_sc = 1.0 / (WS * OS) if fp8o else 1.0
            wq, wk, wv, wo = (get_w("q", suffix), get_w("k", suffix),
                              get_w("v", suffix), get_w("o", suffix, fp8o))

            def start_group(g):
                q0 = g * G
                src_t = load_src(src_pool, xt8v if is_height else x8v, q0)
                xr = None
                if is_height:
                    xr = xst_pool.tile([P, NCB, G, S], f16, tag="xst",
                                       name="xr")
                    nc.sync.dma_start(out=xr,
                                      in_=xt16v[:, :, q0:q0 + G, :])
                qkv = proj_phase(src_t, wq, wk, wv, pools)
                return qkv, xr

            def finish_group(g, ot_full, xr):
                q0 = g * G
                pods = oproj_phase(ot_full, wo, pools, fp8o)
                if is_height:
                    for co in range(NCB):
                        if OH_HW:
                            out_ap = ohsb[co][:, :, q0:q0 + G].rearrange(
                                "p h q -> p q h")
                        else:
                            out_ap = ohsb[co][:, q0:q0 + G, :]
                        ew_stage(
                            STG_ENG[co],
                            out_ap,
                            pods[co].rearrange("p (q s) -> p q s", q=G),
                            xr[:, co], stg_sc)
                else:
                    st = xst_pool.tile([P, NCB, G, S], f16, tag="xst",
                                       name="st")
                    for co in range(NCB):
                        if OH_HW:
                            resid_ap = ohsb[co][:, q0:q0 + G, :]
                        elif HACK_CONTIG:
                            resid_ap = ohsb[co][:, q0:q0 + G, :]
                        else:
                            resid_ap = ohsb[co][:, :, q0:q0 + G].rearrange(
                                "p w i -> p i w")
                        ew_stage(
                            STG_ENG[co],
                            st[:, co],
                            pods[co].rearrange("p (q s) -> p q s", q=G),
                            resid_ap, stg_sc)
                    nc.sync.dma_start(out=out16v[:, :, q0:q0 + G, :],
                                      in_=st)

            qkv, xr = start_group(0)
            for g in range(NG):
                ot_full = attn_phase(qkv, pools, fp8o)
                if g + 1 < NG:
                    nqkv, nxr = start_group(g + 1)
                finish_group(g, ot_full, xr)
                if g + 1 < NG:
                    qkv, xr = nqkv, nxr

        body_ctx = topctx
        src_pool = body_ctx.enter_context(tc.tile_pool(name="src", bufs=2))
        xst_pool = body_ctx.enter_context(tc.tile_pool(name="xst", bufs=2))
        pools = mk_pools(body_ctx, "")

        if timed:
            with tc.For_i(0, reps, 1):
                run_pass(True, src_pool, xst_pool, pools)
                run_pass(False, src_pool, xst_pool, pools)
            nc.sync.dma_start(out=tick, in_=ones_sb[0:1, :])
        else:
            run_pass(True, src_pool, xst_pool, pools)
            run_pass(False, src_pool, xst_pool, pools)

    nc.compile()
    return nc


def _get_program():
    global _PROG
    if _PROG is None:
        _PROG = _build_program()
    return _PROG


def _host_prep(xs, wmap):
    """Per-batch input maps for the SPMD run."""
    xs = np.asarray(xs, dtype=np.float32)
    base = {}
    for n, w in wmap.items():
        wt8 = np.ascontiguousarray(np.asarray(w, dtype=np.float32).T) * WS
        if n.startswith("wo") and not (FP8_O or FP8O_H):
            base[n] = (wt8 / WS).astype(_BF16)
        elif n.startswith("wo") and FP8O_H and not FP8_O:
            # height-pass O fp8, width-pass O bf16: ship both forms
            if n == "wo_h":
                base[n + "8"] = wt8.astype(_F8)
            else:
                base[n] = (wt8 / WS).astype(_BF16)
        else:
            base[n + "8"] = wt8.astype(_F8)

    in_maps = []
    for b in range(NCORES):
        xb = np.ascontiguousarray(xs[b])                        # (C, H, W) f32
        xss = xb * XS
        xT = np.ascontiguousarray(np.swapaxes(xb, 1, 2))        # (C, W, H)
        m = dict(base)
        m["x8"] = xss.astype(_F8)
        m["xt8"] = np.ascontiguousarray(np.swapaxes(xss, 1, 2)).astype(_F8)
        m["xt16"] = xT.astype(np.float16)
        in_maps.append(m)
    return in_maps


def kernel(xs, Wq_h, Wk_h, Wv_h, Wo_h, Wq_w, Wk_w, Wv_w, Wo_w):
    from concourse.bass_utils import run_bass_kernel_spmd

    nc = _get_program()

    wmap = {
        "wq_w": Wq_w, "wk_w": Wk_w, "wv_w": Wv_w, "wo_w": Wo_w,
        "wq_h": Wq_h, "wk_h": Wk_h, "wv_h": Wv_h, "wo_h": Wo_h,
    }
    in_maps = _host_prep(xs, wmap)
    res = run_bass_kernel_spmd(nc, in_maps, core_ids=list(range(NCORES)))
    return np.stack(
        [res.results[b]["out16"].astype(np.float32) for b in range(NCORES)],
        axis=0)
